# revision 17
# baseline (speedup 1.0000x reference)
"""PointNet Feature Propagation kernel for Trainium2 (8 NeuronCores).

Data-parallel over batch B=8: core i owns batch element i end-to-end
(distance matrix, top-3 knn, inverse-distance-weighted feature gather,
2-layer 1x1-conv MLP).  BatchNorm uses *global* batch statistics, so the
per-core partial sums (sum, sumsq per channel) are AllReduce'd across the
8 cores between the matmul and the normalization of each layer.

Device pipeline per core (batch b):
  A) neg-distance matrix via a K=9 augmented matmul:
       neg_d[n,s] = 2*p1.p2 - |p1|^2 - |p2|^2   (largest = nearest)
     64 chunks of [128, 2048] in PSUM; per chunk DVE MAX8 (top-8 values,
     descending) + FIND_INDEX_8 (their column indices).
  B) weights w_k = 1/(d_k+eps) normalized; indices folded into the
     16-partition-wrapped int16 layout dma_gather expects; 16 gather
     groups fetch f2[idx] rows (256 f32 = 1KB) from HBM; gathered rows
     are scaled by w_k (DVE tensor_scalar, per-partition scalar) and
     transposed+summed on the TensorEngine (3 accumulating transposes
     into one PSUM tile) to build interp^T in [channel, n] layout;
     MLP1 = W0 @ [f1; interp]^T accumulated over K=64+128+128.
  C) BN stats via ACT accum_out during PSUM evacuation (+ a Square
     pass), AllReduce, apply scale/bias+ReLU on ACT; MLP2 (K=256);
     second stats/AllReduce/ReLU; DMA out [256, 8192].
"""

import numpy as np

import concourse.bass as bass
import concourse.bacc as bacc
import concourse.mybir as mybir
import concourse.tile as tile
from concourse.bass_utils import run_bass_kernel_spmd

F32 = mybir.dt.float32
U16 = mybir.dt.uint16
I16 = mybir.dt.int16
AF = mybir.ActivationFunctionType
OP = mybir.AluOpType
AX = mybir.AxisListType

B, N, S, D1, D2 = 8, 8192, 2048, 64, 256
C_IN = D1 + D2  # 320
C_OUT = 256
EPS_W = 1e-8
EPS_BN = 1e-5
NCH = N // 128          # 64 chunks of 128 query points
GRP = 4                 # chunks per gather group (= one 512-col MLP1 slab)
NGRP = NCH // GRP       # 16
IDX_PER_GRP = GRP * 3 * 128  # 1536

LAST_RESULTS = None


def build_nc(n_cores: int = 8, debug: bool = False,
             stop_after: str | None = None, reps: int = 1) -> bass.Bass:
    """stop_after in {None, 'A', 'fold', 'gather', 'mlp1', 'bn1'} truncates
    the pipeline for HW bisection (output left partially zero)."""
    nc = bacc.Bacc("TRN2", debug=debug, num_devices=n_cores)

    a_d = nc.dram_tensor("a_aug", [9, N], F32, kind="ExternalInput")
    b_d = nc.dram_tensor("b_aug", [9, S], F32, kind="ExternalInput")
    p1f_d = nc.dram_tensor("p1feat", [D1, N], F32, kind="ExternalInput")
    f2_d = nc.dram_tensor("f2", [S, D2], F32, kind="ExternalInput")
    w0t_d = nc.dram_tensor("W0T", [C_IN, C_OUT], F32, kind="ExternalInput")
    w1t_d = nc.dram_tensor("W1T", [C_OUT, C_OUT], F32, kind="ExternalInput")
    g0_d = nc.dram_tensor("g0", [128, 2], F32, kind="ExternalInput")
    be0_d = nc.dram_tensor("be0", [128, 2], F32, kind="ExternalInput")
    g1_d = nc.dram_tensor("g1", [128, 2], F32, kind="ExternalInput")
    be1_d = nc.dram_tensor("be1", [128, 2], F32, kind="ExternalInput")
    id_d = nc.dram_tensor("ident", [128, 128], F32, kind="ExternalInput")
    out_d = nc.dram_tensor("out", [C_OUT, N], F32, kind="ExternalOutput")

    inv_cnt = 1.0 / float(n_cores * N)
    groups = [list(range(n_cores))]

    for _ in range(reps):
        _build_body(nc, n_cores, inv_cnt, groups, stop_after,
                    a_d, b_d, p1f_d, f2_d, w0t_d, w1t_d,
                    g0_d, be0_d, g1_d, be1_d, id_d, out_d)
    nc.compile()  # Bacc pass pipeline (library + ACT table loads, EVSEM split)
    return nc


_STOP_LEVEL = {"A": 0, "fold": 1, "gonly": 2, "gather": 2.5,
               "mlp1": 3, "bn1": 4, None: 99}


def _build_body(nc, n_cores, inv_cnt, groups, stop_after,
                a_d, b_d, p1f_d, f2_d, w0t_d, w1t_d,
                g0_d, be0_d, g1_d, be1_d, id_d, out_d):
    level = _STOP_LEVEL[stop_after]
    with tile.TileContext(nc) as tc:
        with (
            tc.tile_pool(name="persist", bufs=1) as pp,
            tc.tile_pool(name="dram", bufs=1, space="DRAM") as dp,
        ):
            ident = pp.tile([128, 128], F32, tag="ident")
            nc.sync.dma_start(out=ident[:, :], in_=id_d[:, :])

            vals = pp.tile([128, NCH * 8], F32, tag="vals")      # top-8 neg-d
            idxs = pp.tile([128, NCH * 8], U16, tag="idxs")      # their cols
            idx16 = pp.tile([128, NCH * 3 * 8], I16, tag="idx16")
            dtmp = pp.tile([128, NCH * 3], F32, tag="dtmp")
            w_un = pp.tile([128, NCH * 3], F32, tag="w_un")
            wsum = pp.tile([128, NCH], F32, tag="wsum")
            wsr = pp.tile([128, NCH], F32, tag="wsr")
            wn = pp.tile([128, NCH * 3], F32, tag="wn")

            w0a = pp.tile([D1, C_OUT], F32, tag="w0a")
            w0b = pp.tile([128, C_OUT], F32, tag="w0b")
            w0c = pp.tile([128, C_OUT], F32, tag="w0c")
            w1a = pp.tile([128, C_OUT], F32, tag="w1a")
            w1b = pp.tile([128, C_OUT], F32, tag="w1b")
            gam0 = pp.tile([128, 2], F32, tag="gam0")
            bet0 = pp.tile([128, 2], F32, tag="bet0")
            gam1 = pp.tile([128, 2], F32, tag="gam1")
            bet1 = pp.tile([128, 2], F32, tag="bet1")
            nc.sync.dma_start(out=w0a[:, :], in_=w0t_d[0:D1, :])
            nc.sync.dma_start(out=w0b[:, :], in_=w0t_d[D1:D1 + 128, :])
            nc.sync.dma_start(out=w0c[:, :], in_=w0t_d[D1 + 128:C_IN, :])
            nc.sync.dma_start(out=w1a[:, :], in_=w1t_d[0:128, :])
            nc.sync.dma_start(out=w1b[:, :], in_=w1t_d[128:256, :])
            nc.sync.dma_start(out=gam0[:, :], in_=g0_d[:, :])
            nc.sync.dma_start(out=bet0[:, :], in_=be0_d[:, :])
            nc.sync.dma_start(out=gam1[:, :], in_=g1_d[:, :])
            nc.sync.dma_start(out=bet1[:, :], in_=be1_d[:, :])

            # pre-BN layer-1 activations, [channel, n] layout, 2 tiles
            out1 = [pp.tile([128, N], F32, tag=f"out1_{t}", name=f"out1_{t}") for t in range(2)]
            s1 = [pp.tile([128, NGRP], F32, tag=f"s1_{t}", name=f"s1_{t}") for t in range(2)]
            s1q = [pp.tile([128, NGRP], F32, tag=f"s1q_{t}", name=f"s1q_{t}") for t in range(2)]
            s2 = [pp.tile([128, NGRP], F32, tag=f"s2_{t}", name=f"s2_{t}") for t in range(2)]
            s2q = [pp.tile([128, NGRP], F32, tag=f"s2q_{t}", name=f"s2q_{t}") for t in range(2)]

            # ---------------- phase A: distances + top-3 ----------------
            with (
                tc.tile_pool(name="pA", bufs=1) as pa,
                tc.tile_pool(name="pdist", bufs=2, space="PSUM") as pd_pool,
            ):
                # host-prepped augmented coords: one clean load each
                aT = pa.tile([9, N], F32, tag="aT")
                bT = pa.tile([9, S], F32, tag="bT")
                nc.sync.dma_start(out=aT[:, :], in_=a_d[:, :])
                nc.sync.dma_start(out=bT[:, :], in_=b_d[:, :])

                for c in range(NCH):
                    pd = pd_pool.tile([128, S], F32, tag="pd", name="pd")
                    for j in range(4):
                        nc.tensor.matmul(
                            pd[:, 512 * j:512 * (j + 1)],
                            lhsT=aT[:, 128 * c:128 * (c + 1)],
                            rhs=bT[:, 512 * j:512 * (j + 1)],
                            start=True, stop=True,
                        )
                    nc.vector.max(vals[:, 8 * c:8 * c + 8], pd[:, :])
                    nc.vector.max_index(
                        idxs[:, 8 * c:8 * c + 8], vals[:, 8 * c:8 * c + 8],
                        pd[:, :],
                    )

            # ---------------- weights + index fold ----------------
            if level < 1:
                nc.sync.dma_start(out=out_d[0:128, 0:512], in_=vals[:, :])
                return
            v3 = vals[:, :].rearrange("p (c e) -> p c e", e=8)[:, :, 0:3]
            d3 = dtmp[:, :].rearrange("p (c e) -> p c e", e=3)
            # d = -negd + eps_w
            nc.vector.tensor_scalar(d3, v3, -1.0, EPS_W, OP.mult, OP.add)
            nc.vector.reciprocal(w_un[:, :], dtmp[:, :])
            u3 = w_un[:, :].rearrange("p (c e) -> p c e", e=3)
            nc.vector.tensor_reduce(wsum[:, :], u3, axis=AX.X, op=OP.add)
            nc.vector.reciprocal(wsr[:, :], wsum[:, :])
            n3 = wn[:, :].rearrange("p (c e) -> p c e", e=3)
            for k in range(3):
                nc.vector.tensor_tensor(
                    n3[:, :, k], u3[:, :, k], wsr[:, :], OP.mult
                )

            # fold idxs [128, (c,8)] -> idx16 [16, (c,3,8)] wrapped layout
            idx_i16 = idxs[:, :].bitcast(I16)
            src3 = idx_i16.rearrange("p (c e) -> p c e", e=8)
            dst4 = idx16[:, :].rearrange("p (c k e) -> p c k e", k=3, e=8)
            for r in range(8):
                for k in range(3):
                    nc.sync.dma_start(
                        out=dst4[0:16, :, k, r],
                        in_=src3[16 * r:16 * (r + 1), :, k],
                    )
            for m in range(1, 8):
                nc.sync.dma_start(
                    out=idx16[16 * m:16 * (m + 1), :], in_=idx16[0:16, :]
                )

            if level < 2:
                nc.gpsimd.dma_start(out=out_d[0:128, 0:384],
                                    in_=idx16[:, :].bitcast(U16)[:, 0:384])
                return

            # ---------- phase B: gather, interp^T, MLP1 ----------
            do_mlp1 = level >= 3
            with (
                tc.tile_pool(name="pB", bufs=1) as pb,
                tc.tile_pool(name="pBg", bufs=2) as pbg,
                tc.tile_pool(name="pBs", bufs=6) as pbs,
                tc.tile_pool(name="pBi", bufs=2) as pbi,
                tc.tile_pool(name="pBq", bufs=2) as pbq,
                tc.tile_pool(name="ptx", bufs=4, space="PSUM") as ptx,
                tc.tile_pool(name="pm1", bufs=2, space="PSUM") as pm1,
            ):
                p1f = pb.tile([D1, N], F32, tag="p1f")
                nc.sync.dma_start(out=p1f[:, :], in_=p1f_d[:, :])

                for g in range(NGRP):
                    gbuf = pbg.tile([128, GRP * 3, D2], F32, tag="gbuf", name="gbuf")
                    nc.gpsimd.dma_gather(
                        out_ap=gbuf[:, :, :],
                        in_ap=f2_d[:, :],
                        idxs_ap=idx16[:, 96 * g:96 * (g + 1)],
                        num_idxs=IDX_PER_GRP,
                        num_idxs_reg=IDX_PER_GRP,
                        elem_size=D2,
                        single_packet=False,
                    )
                    irot = [pbi.tile([128, 512], F32, tag=f"irot{h}", name=f"irot{h}")
                            for h in range(2)]
                    if level == 2:
                        nc.vector.tensor_scalar_mul(
                            wn[:, 3 * GRP * g:3 * GRP * (g + 1)],
                            gbuf[:, :, 0], 1.0)
                        continue
                    for cl in range(GRP):
                        gs = []
                        for k in range(3):
                            gsk = pbs.tile([128, D2], F32, tag="gs", name="gs")
                            col = (GRP * g + cl) * 3 + k
                            nc.vector.tensor_scalar_mul(
                                gsk[:, :], gbuf[:, cl * 3 + k, :],
                                wn[:, col:col + 1],
                            )
                            gs.append(gsk)
                        for h in range(2):
                            pt = ptx.tile([128, 128], F32, tag="pt", name="pt")
                            for k in range(3):
                                nc.tensor.matmul(
                                    pt[:, :],
                                    lhsT=gs[k][:, 128 * h:128 * (h + 1)],
                                    rhs=ident[:, :],
                                    is_transpose=True,
                                    start=(k == 0), stop=(k == 2),
                                )
                            nc.scalar.copy(
                                irot[h][:, 128 * cl:128 * (cl + 1)], pt[:, :]
                            )
                    for ot in range(2 if do_mlp1 else 0):
                        pm = pm1.tile([128, 512], F32, tag="pm", name="pm")
                        osl = slice(128 * ot, 128 * (ot + 1))
                        nsl = slice(512 * g, 512 * (g + 1))
                        nc.tensor.matmul(pm[:, :], lhsT=w0a[:, osl],
                                         rhs=p1f[:, nsl],
                                         start=True, stop=False)
                        nc.tensor.matmul(pm[:, :], lhsT=w0b[:, osl],
                                         rhs=irot[0][:, :],
                                         start=False, stop=False)
                        nc.tensor.matmul(pm[:, :], lhsT=w0c[:, osl],
                                         rhs=irot[1][:, :],
                                         start=False, stop=True)
                        nc.scalar.activation(
                            out1[ot][:, nsl], pm[:, :], AF.Copy,
                            accum_out=s1[ot][:, g:g + 1],
                        )
                        sqd = pbq.tile([128, 512], F32, tag="sqd", name="sqd")
                        nc.scalar.activation(
                            sqd[:, :], pm[:, :], AF.Square,
                            accum_out=s1q[ot][:, g:g + 1],
                        )

            if level < 3:
                return

            # ---------------- BN1: allreduce + apply ----------------
            if level < 4:
                nc.sync.dma_start(out=out_d[0:128, :], in_=out1[0][:, :])
                return
            ar_in1 = pp.tile([128, 4], F32, tag="ar_in1")
            ar_out1 = pp.tile([128, 4], F32, tag="ar_out1")
            for t in range(2):
                nc.vector.tensor_reduce(ar_in1[:, t:t + 1], s1[t][:, :],
                                        axis=AX.X, op=OP.add)
                nc.vector.tensor_reduce(ar_in1[:, 2 + t:3 + t], s1q[t][:, :],
                                        axis=AX.X, op=OP.add)
            bnc_i1 = dp.tile([128, 4], F32, tag="bnc_i1")
            bnc_o1 = dp.tile([128, 4], F32, tag="bnc_o1")
            nc.sync.dma_start(out=bnc_i1[:, :], in_=ar_in1[:, :])
            nc.gpsimd.collective_compute(
                "AllReduce", OP.add, replica_groups=groups,
                ins=[bnc_i1[:, :].opt()], outs=[bnc_o1[:, :].opt()],
            )
            nc.sync.dma_start(out=ar_out1[:, :], in_=bnc_o1[:, :])

            def bn_scale_bias(ar_out, gam, bet, tag):
                mu = pp.tile([128, 2], F32, tag=f"mu{tag}", name=f"mu{tag}")
                ex2 = pp.tile([128, 2], F32, tag=f"ex2{tag}", name=f"ex2{tag}")
                var = pp.tile([128, 2], F32, tag=f"var{tag}", name=f"var{tag}")
                sd = pp.tile([128, 2], F32, tag=f"sd{tag}", name=f"sd{tag}")
                rs = pp.tile([128, 2], F32, tag=f"rs{tag}", name=f"rs{tag}")
                sc = pp.tile([128, 2], F32, tag=f"sc{tag}", name=f"sc{tag}")
                msc = pp.tile([128, 2], F32, tag=f"msc{tag}", name=f"msc{tag}")
                bi = pp.tile([128, 2], F32, tag=f"bi{tag}", name=f"bi{tag}")
                nc.vector.tensor_scalar_mul(mu[:, :], ar_out[:, 0:2], inv_cnt)
                nc.vector.tensor_scalar_mul(ex2[:, :], ar_out[:, 2:4], inv_cnt)
                nc.vector.tensor_tensor(var[:, :], mu[:, :], mu[:, :], OP.mult)
                nc.vector.tensor_tensor(var[:, :], ex2[:, :], var[:, :],
                                        OP.subtract)
                epst = pp.tile([128, 1], F32, tag=f"eps{tag}", name=f"eps{tag}")
                nc.vector.memset(epst[:, :], EPS_BN)
                nc.scalar.activation(sd[:, :], var[:, :], AF.Sqrt,
                                     bias=epst[:, :])
                nc.vector.reciprocal(rs[:, :], sd[:, :])
                nc.vector.tensor_tensor(sc[:, :], rs[:, :], gam[:, :], OP.mult)
                nc.vector.tensor_tensor(msc[:, :], mu[:, :], sc[:, :], OP.mult)
                nc.vector.tensor_tensor(bi[:, :], bet[:, :], msc[:, :],
                                        OP.subtract)
                return sc, bi

            sc1, bi1 = bn_scale_bias(ar_out1, gam0, bet0, "1")
            for t in range(2):
                nc.scalar.activation(
                    out1[t][:, :], out1[t][:, :], AF.Relu,
                    bias=bi1[:, t:t + 1], scale=sc1[:, t:t + 1],
                )

            if level < 99:
                nc.sync.dma_start(out=out_d[0:128, :], in_=out1[0][:, :])
                return

            # ---------------- phase C: MLP2 + BN2 + out ----------------
            with (
                tc.tile_pool(name="pC", bufs=1) as pc,
                tc.tile_pool(name="pCq", bufs=2) as pcq,
                tc.tile_pool(name="pm2", bufs=2, space="PSUM") as pm2_pool,
            ):
                y2 = [pc.tile([128, N], F32, tag=f"y2_{t}", name=f"y2_{t}") for t in range(2)]
                for g in range(NGRP):
                    nsl = slice(512 * g, 512 * (g + 1))
                    for ot in range(2):
                        osl = slice(128 * ot, 128 * (ot + 1))
                        pm = pm2_pool.tile([128, 512], F32, tag="pm2", name="pm2")
                        nc.tensor.matmul(pm[:, :], lhsT=w1a[:, osl],
                                         rhs=out1[0][:, nsl],
                                         start=True, stop=False)
                        nc.tensor.matmul(pm[:, :], lhsT=w1b[:, osl],
                                         rhs=out1[1][:, nsl],
                                         start=False, stop=True)
                        nc.scalar.activation(
                            y2[ot][:, nsl], pm[:, :], AF.Copy,
                            accum_out=s2[ot][:, g:g + 1],
                        )
                        sqd = pcq.tile([128, 512], F32, tag="sqd2", name="sqd2")
                        nc.scalar.activation(
                            sqd[:, :], pm[:, :], AF.Square,
                            accum_out=s2q[ot][:, g:g + 1],
                        )

                ar_in2 = pp.tile([128, 4], F32, tag="ar_in2")
                ar_out2 = pp.tile([128, 4], F32, tag="ar_out2")
                for t in range(2):
                    nc.vector.tensor_reduce(ar_in2[:, t:t + 1], s2[t][:, :],
                                            axis=AX.X, op=OP.add)
                    nc.vector.tensor_reduce(ar_in2[:, 2 + t:3 + t],
                                            s2q[t][:, :], axis=AX.X, op=OP.add)
                bnc_i2 = dp.tile([128, 4], F32, tag="bnc_i2")
                bnc_o2 = dp.tile([128, 4], F32, tag="bnc_o2")
                nc.sync.dma_start(out=bnc_i2[:, :], in_=ar_in2[:, :])
                nc.gpsimd.collective_compute(
                    "AllReduce", OP.add, replica_groups=groups,
                    ins=[bnc_i2[:, :].opt()], outs=[bnc_o2[:, :].opt()],
                )
                nc.sync.dma_start(out=ar_out2[:, :], in_=bnc_o2[:, :])
                sc2, bi2 = bn_scale_bias(ar_out2, gam1, bet1, "2")
                for t in range(2):
                    nc.scalar.activation(
                        y2[t][:, :], y2[t][:, :], AF.Relu,
                        bias=bi2[:, t:t + 1], scale=sc2[:, t:t + 1],
                    )
                    nc.sync.dma_start(
                        out=out_d[128 * t:128 * (t + 1), :], in_=y2[t][:, :]
                    )


def make_in_maps(xyz_pc1, xyz_pc2, points1, points2,
                 W0, b0, gamma0, beta0, W1, b1, gamma1, beta1,
                 n_cores=8):
    # Conv bias b0/b1 cancels exactly in train-mode BatchNorm; unused.
    del b0, b1
    w0t = np.ascontiguousarray(W0.T.astype(np.float32))
    w1t = np.ascontiguousarray(W1.T.astype(np.float32))
    g0 = np.ascontiguousarray(gamma0.reshape(2, 128).T.astype(np.float32))
    be0 = np.ascontiguousarray(beta0.reshape(2, 128).T.astype(np.float32))
    g1 = np.ascontiguousarray(gamma1.reshape(2, 128).T.astype(np.float32))
    be1 = np.ascontiguousarray(beta1.reshape(2, 128).T.astype(np.float32))
    ident = np.eye(128, dtype=np.float32)
    in_maps = []
    for b in range(n_cores):
        x1 = xyz_pc1[b].astype(np.float32)
        x2 = xyz_pc2[b].astype(np.float32)
        a_aug = np.concatenate(
            [x1, x1 * x1, np.full((3, x1.shape[1]), -1.0, np.float32)], axis=0)
        b_aug = np.concatenate(
            [2.0 * x2, np.full((3, x2.shape[1]), -1.0, np.float32), x2 * x2],
            axis=0)
        in_maps.append({
            "a_aug": np.ascontiguousarray(a_aug),
            "b_aug": np.ascontiguousarray(b_aug),
            "p1feat": np.ascontiguousarray(points1[b].astype(np.float32)),
            "f2": np.ascontiguousarray(points2[b].T.astype(np.float32)),
            "W0T": w0t, "W1T": w1t,
            "g0": g0, "be0": be0, "g1": g1, "be1": be1,
            "ident": ident,
        })
    return in_maps


def kernel(**inputs) -> np.ndarray:
    global LAST_RESULTS
    in_maps = make_in_maps(
        inputs["xyz_pc1"], inputs["xyz_pc2"],
        inputs["points1"], inputs["points2"],
        inputs["W0"], inputs["b0"], inputs["gamma0"], inputs["beta0"],
        inputs["W1"], inputs["b1"], inputs["gamma1"], inputs["beta1"],
        n_cores=B,
    )
    nc = build_nc(n_cores=B, debug=False)
    res = run_bass_kernel_spmd(nc, in_maps, core_ids=list(range(B)))
    LAST_RESULTS = res
    out = np.stack([res.results[b]["out"] for b in range(B)], axis=0)
    return out.astype(np.float32)


# revision 18
# speedup vs baseline: 230.8031x; 230.8031x over previous
"""PointNet Feature Propagation kernel for Trainium2 (8 NeuronCores).

Data-parallel over batch B=8: core i owns batch element i end-to-end
(distance matrix, top-3 knn, inverse-distance-weighted feature gather,
2-layer 1x1-conv MLP).  BatchNorm uses *global* batch statistics, so the
per-core partial sums (sum, sumsq per channel) are AllReduce'd across the
8 cores between the matmul and the normalization of each layer.

Device pipeline per core (batch b):
  A) neg-distance matrix via a K=9 augmented matmul:
       neg_d[n,s] = 2*p1.p2 - |p1|^2 - |p2|^2   (largest = nearest)
     64 chunks of [128, 2048] in PSUM; per chunk DVE MAX8 (top-8 values,
     descending) + FIND_INDEX_8 (their column indices).
  B) weights w_k = 1/(d_k+eps) normalized; indices folded into the
     16-partition-wrapped int16 layout dma_gather expects; 16 gather
     groups fetch f2[idx] rows (256 f32 = 1KB) from HBM; gathered rows
     are scaled by w_k (DVE tensor_scalar, per-partition scalar) and
     transposed+summed on the TensorEngine (3 accumulating transposes
     into one PSUM tile) to build interp^T in [channel, n] layout;
     MLP1 = W0 @ [f1; interp]^T accumulated over K=64+128+128.
  C) BN stats via ACT accum_out during PSUM evacuation (+ a Square
     pass), AllReduce, apply scale/bias+ReLU on ACT; MLP2 (K=256);
     second stats/AllReduce/ReLU; DMA out [256, 8192].
"""

import numpy as np

import concourse.bass as bass
import concourse.bacc as bacc
import concourse.mybir as mybir
import concourse.tile as tile
from concourse.bass_utils import run_bass_kernel_spmd

F32 = mybir.dt.float32
U16 = mybir.dt.uint16
I16 = mybir.dt.int16
AF = mybir.ActivationFunctionType
OP = mybir.AluOpType
AX = mybir.AxisListType

B, N, S, D1, D2 = 8, 8192, 2048, 64, 256
C_IN = D1 + D2  # 320
C_OUT = 256
EPS_W = 1e-8
EPS_BN = 1e-5
NCH = N // 128          # 64 chunks of 128 query points
GRP = 4                 # chunks per gather group (= one 512-col MLP1 slab)
NGRP = NCH // GRP       # 16
IDX_PER_GRP = GRP * 3 * 128  # 1536

LAST_RESULTS = None


def build_nc(n_cores: int = 8, debug: bool = False,
             stop_after: str | None = None, reps: int = 1) -> bass.Bass:
    """stop_after in {None, 'A', 'fold', 'gather', 'mlp1', 'bn1'} truncates
    the pipeline for HW bisection (output left partially zero)."""
    nc = bacc.Bacc("TRN2", debug=debug, num_devices=n_cores)

    a_d = nc.dram_tensor("a_aug", [9, N], F32, kind="ExternalInput")
    b_d = nc.dram_tensor("b_aug", [9, S], F32, kind="ExternalInput")
    p1f_d = nc.dram_tensor("p1feat", [D1, N], F32, kind="ExternalInput")
    f2_d = nc.dram_tensor("f2", [S, D2], F32, kind="ExternalInput")
    w0t_d = nc.dram_tensor("W0T", [C_IN, C_OUT], F32, kind="ExternalInput")
    w1t_d = nc.dram_tensor("W1T", [C_OUT, C_OUT], F32, kind="ExternalInput")
    g0_d = nc.dram_tensor("g0", [128, 2], F32, kind="ExternalInput")
    be0_d = nc.dram_tensor("be0", [128, 2], F32, kind="ExternalInput")
    g1_d = nc.dram_tensor("g1", [128, 2], F32, kind="ExternalInput")
    be1_d = nc.dram_tensor("be1", [128, 2], F32, kind="ExternalInput")
    id_d = nc.dram_tensor("ident", [128, 128], F32, kind="ExternalInput")
    out_d = nc.dram_tensor("out", [C_OUT, N], F32, kind="ExternalOutput")

    inv_cnt = 1.0 / float(n_cores * N)
    groups = [list(range(n_cores))]

    for _ in range(reps):
        _build_body(nc, n_cores, inv_cnt, groups, stop_after,
                    a_d, b_d, p1f_d, f2_d, w0t_d, w1t_d,
                    g0_d, be0_d, g1_d, be1_d, id_d, out_d)
    nc.compile()  # Bacc pass pipeline (library + ACT table loads, EVSEM split)
    return nc


_STOP_LEVEL = {"A": 0, "fold": 1, "gonly": 2, "gather": 2.5,
               "mlp1": 3, "bn1": 4, None: 99}


def _build_body(nc, n_cores, inv_cnt, groups, stop_after,
                a_d, b_d, p1f_d, f2_d, w0t_d, w1t_d,
                g0_d, be0_d, g1_d, be1_d, id_d, out_d):
    level = _STOP_LEVEL[stop_after]
    with tile.TileContext(nc) as tc:
        with (
            tc.tile_pool(name="persist", bufs=1) as pp,
            tc.tile_pool(name="dram", bufs=1, space="DRAM") as dp,
        ):
            ident = pp.tile([128, 128], F32, tag="ident")
            nc.sync.dma_start(out=ident[:, :], in_=id_d[:, :])

            vals = pp.tile([128, NCH * 8], F32, tag="vals")      # top-8 neg-d
            idxs = pp.tile([128, NCH * 8], U16, tag="idxs")      # their cols
            idx16 = pp.tile([128, NCH * 3 * 8], I16, tag="idx16")
            dtmp = pp.tile([128, NCH * 3], F32, tag="dtmp")
            w_un = pp.tile([128, NCH * 3], F32, tag="w_un")
            wsum = pp.tile([128, NCH], F32, tag="wsum")
            wsr = pp.tile([128, NCH], F32, tag="wsr")
            wn = pp.tile([128, NCH * 3], F32, tag="wn")

            w0a = pp.tile([D1, C_OUT], F32, tag="w0a")
            w0b = pp.tile([128, C_OUT], F32, tag="w0b")
            w0c = pp.tile([128, C_OUT], F32, tag="w0c")
            w1a = pp.tile([128, C_OUT], F32, tag="w1a")
            w1b = pp.tile([128, C_OUT], F32, tag="w1b")
            gam0 = pp.tile([128, 2], F32, tag="gam0")
            bet0 = pp.tile([128, 2], F32, tag="bet0")
            gam1 = pp.tile([128, 2], F32, tag="gam1")
            bet1 = pp.tile([128, 2], F32, tag="bet1")
            nc.sync.dma_start(out=w0a[:, :], in_=w0t_d[0:D1, :])
            nc.sync.dma_start(out=w0b[:, :], in_=w0t_d[D1:D1 + 128, :])
            nc.sync.dma_start(out=w0c[:, :], in_=w0t_d[D1 + 128:C_IN, :])
            nc.sync.dma_start(out=w1a[:, :], in_=w1t_d[0:128, :])
            nc.sync.dma_start(out=w1b[:, :], in_=w1t_d[128:256, :])
            nc.sync.dma_start(out=gam0[:, :], in_=g0_d[:, :])
            nc.sync.dma_start(out=bet0[:, :], in_=be0_d[:, :])
            nc.sync.dma_start(out=gam1[:, :], in_=g1_d[:, :])
            nc.sync.dma_start(out=bet1[:, :], in_=be1_d[:, :])

            # pre-BN layer-1 activations, [channel, n] layout, 2 tiles
            out1 = [pp.tile([128, N], F32, tag=f"out1_{t}", name=f"out1_{t}") for t in range(2)]
            s1 = [pp.tile([128, NGRP], F32, tag=f"s1_{t}", name=f"s1_{t}") for t in range(2)]
            s1q = [pp.tile([128, NGRP], F32, tag=f"s1q_{t}", name=f"s1q_{t}") for t in range(2)]
            s2 = [pp.tile([128, NGRP], F32, tag=f"s2_{t}", name=f"s2_{t}") for t in range(2)]
            s2q = [pp.tile([128, NGRP], F32, tag=f"s2q_{t}", name=f"s2q_{t}") for t in range(2)]

            # ---------------- phase A: distances + top-3 ----------------
            with (
                tc.tile_pool(name="pA", bufs=1) as pa,
                tc.tile_pool(name="pdist", bufs=2, space="PSUM") as pd_pool,
            ):
                # host-prepped augmented coords: one clean load each
                aT = pa.tile([9, N], F32, tag="aT")
                bT = pa.tile([9, S], F32, tag="bT")
                nc.sync.dma_start(out=aT[:, :], in_=a_d[:, :])
                nc.sync.dma_start(out=bT[:, :], in_=b_d[:, :])

                for c in range(NCH):
                    pd = pd_pool.tile([128, S], F32, tag="pd", name="pd")
                    for j in range(4):
                        nc.tensor.matmul(
                            pd[:, 512 * j:512 * (j + 1)],
                            lhsT=aT[:, 128 * c:128 * (c + 1)],
                            rhs=bT[:, 512 * j:512 * (j + 1)],
                            start=True, stop=True,
                        )
                    # evacuate to SBUF on the (otherwise idle) ACT engine:
                    # frees the PSUM bank quickly and lets MAX8 run as a
                    # single-src SBUF op (2x DVE perf-mode eligible).
                    dsb = pa.tile([128, S], F32, tag="dsb", name="dsb",
                                  bufs=3)
                    nc.scalar.copy(dsb[:, :], pd[:, :])
                    nc.vector.max(vals[:, 8 * c:8 * c + 8], dsb[:, :])
                    nc.vector.max_index(
                        idxs[:, 8 * c:8 * c + 8], vals[:, 8 * c:8 * c + 8],
                        dsb[:, :],
                    )

            # ---------------- weights + index fold ----------------
            if level < 1:
                nc.sync.dma_start(out=out_d[0:128, 0:512], in_=vals[:, :])
                return
            v3 = vals[:, :].rearrange("p (c e) -> p c e", e=8)[:, :, 0:3]
            d3 = dtmp[:, :].rearrange("p (c e) -> p c e", e=3)
            # d = -negd + eps_w
            nc.vector.tensor_scalar(d3, v3, -1.0, EPS_W, OP.mult, OP.add)
            nc.vector.reciprocal(w_un[:, :], dtmp[:, :])
            u3 = w_un[:, :].rearrange("p (c e) -> p c e", e=3)
            nc.vector.tensor_reduce(wsum[:, :], u3, axis=AX.X, op=OP.add)
            nc.vector.reciprocal(wsr[:, :], wsum[:, :])
            n3 = wn[:, :].rearrange("p (c e) -> p c e", e=3)
            for k in range(3):
                nc.vector.tensor_tensor(
                    n3[:, :, k], u3[:, :, k], wsr[:, :], OP.mult
                )

            # fold idxs [128, (c,8)] -> idx16 [16, (c,3,8)] wrapped layout
            idx_i16 = idxs[:, :].bitcast(I16)
            src3 = idx_i16.rearrange("p (c e) -> p c e", e=8)
            dst4 = idx16[:, :].rearrange("p (c k e) -> p c k e", k=3, e=8)
            for r in range(8):
                for k in range(3):
                    nc.sync.dma_start(
                        out=dst4[0:16, :, k, r],
                        in_=src3[16 * r:16 * (r + 1), :, k],
                    )
            for m in range(1, 8):
                nc.sync.dma_start(
                    out=idx16[16 * m:16 * (m + 1), :], in_=idx16[0:16, :]
                )

            if level < 2:
                nc.gpsimd.dma_start(out=out_d[0:128, 0:384],
                                    in_=idx16[:, :].bitcast(U16)[:, 0:384])
                return

            # ---------- phase B: gather, interp^T, MLP1 ----------
            do_mlp1 = level >= 3
            with (
                tc.tile_pool(name="pB", bufs=1) as pb,
                tc.tile_pool(name="pBg", bufs=2) as pbg,
                tc.tile_pool(name="pBs", bufs=6) as pbs,
                tc.tile_pool(name="pBi", bufs=2) as pbi,
                tc.tile_pool(name="pBq", bufs=2) as pbq,
                tc.tile_pool(name="ptx", bufs=4, space="PSUM") as ptx,
                tc.tile_pool(name="pm1", bufs=2, space="PSUM") as pm1,
            ):
                p1f = pb.tile([D1, N], F32, tag="p1f")
                nc.sync.dma_start(out=p1f[:, :], in_=p1f_d[:, :])

                for g in range(NGRP):
                    gbuf = pbg.tile([128, GRP * 3, D2], F32, tag="gbuf", name="gbuf")
                    nc.gpsimd.dma_gather(
                        out_ap=gbuf[:, :, :],
                        in_ap=f2_d[:, :],
                        idxs_ap=idx16[:, 96 * g:96 * (g + 1)],
                        num_idxs=IDX_PER_GRP,
                        num_idxs_reg=IDX_PER_GRP,
                        elem_size=D2,
                        single_packet=False,
                    )
                    irot = [pbi.tile([128, 512], F32, tag=f"irot{h}", name=f"irot{h}")
                            for h in range(2)]
                    if level == 2:
                        nc.vector.tensor_scalar_mul(
                            wn[:, 3 * GRP * g:3 * GRP * (g + 1)],
                            gbuf[:, :, 0], 1.0)
                        continue
                    for cl in range(GRP):
                        gs = []
                        for k in range(3):
                            gsk = pbs.tile([128, D2], F32, tag="gs", name="gs")
                            col = (GRP * g + cl) * 3 + k
                            nc.vector.tensor_scalar_mul(
                                gsk[:, :], gbuf[:, cl * 3 + k, :],
                                wn[:, col:col + 1],
                            )
                            gs.append(gsk)
                        for h in range(2):
                            pt = ptx.tile([128, 128], F32, tag="pt", name="pt")
                            for k in range(3):
                                nc.tensor.matmul(
                                    pt[:, :],
                                    lhsT=gs[k][:, 128 * h:128 * (h + 1)],
                                    rhs=ident[:, :],
                                    is_transpose=True,
                                    start=(k == 0), stop=(k == 2),
                                )
                            nc.scalar.copy(
                                irot[h][:, 128 * cl:128 * (cl + 1)], pt[:, :]
                            )
                    for ot in range(2 if do_mlp1 else 0):
                        pm = pm1.tile([128, 512], F32, tag="pm", name="pm")
                        osl = slice(128 * ot, 128 * (ot + 1))
                        nsl = slice(512 * g, 512 * (g + 1))
                        nc.tensor.matmul(pm[:, :], lhsT=w0a[:, osl],
                                         rhs=p1f[:, nsl],
                                         start=True, stop=False)
                        nc.tensor.matmul(pm[:, :], lhsT=w0b[:, osl],
                                         rhs=irot[0][:, :],
                                         start=False, stop=False)
                        nc.tensor.matmul(pm[:, :], lhsT=w0c[:, osl],
                                         rhs=irot[1][:, :],
                                         start=False, stop=True)
                        nc.scalar.activation(
                            out1[ot][:, nsl], pm[:, :], AF.Copy,
                            accum_out=s1[ot][:, g:g + 1],
                        )
                        sqd = pbq.tile([128, 512], F32, tag="sqd", name="sqd")
                        nc.scalar.activation(
                            sqd[:, :], pm[:, :], AF.Square,
                            accum_out=s1q[ot][:, g:g + 1],
                        )

            if level < 3:
                return

            # ---------------- BN1: allreduce + apply ----------------
            if level < 4:
                nc.sync.dma_start(out=out_d[0:128, :], in_=out1[0][:, :])
                return
            ar_in1 = pp.tile([128, 4], F32, tag="ar_in1")
            ar_out1 = pp.tile([128, 4], F32, tag="ar_out1")
            for t in range(2):
                nc.vector.tensor_reduce(ar_in1[:, t:t + 1], s1[t][:, :],
                                        axis=AX.X, op=OP.add)
                nc.vector.tensor_reduce(ar_in1[:, 2 + t:3 + t], s1q[t][:, :],
                                        axis=AX.X, op=OP.add)
            bnc_i1 = dp.tile([128, 4], F32, tag="bnc_i1")
            bnc_o1 = dp.tile([128, 4], F32, tag="bnc_o1")
            nc.sync.dma_start(out=bnc_i1[:, :], in_=ar_in1[:, :])
            nc.gpsimd.collective_compute(
                "AllReduce", OP.add, replica_groups=groups,
                ins=[bnc_i1[:, :].opt()], outs=[bnc_o1[:, :].opt()],
            )
            nc.sync.dma_start(out=ar_out1[:, :], in_=bnc_o1[:, :])

            def bn_scale_bias(ar_out, gam, bet, tag):
                mu = pp.tile([128, 2], F32, tag=f"mu{tag}", name=f"mu{tag}")
                ex2 = pp.tile([128, 2], F32, tag=f"ex2{tag}", name=f"ex2{tag}")
                var = pp.tile([128, 2], F32, tag=f"var{tag}", name=f"var{tag}")
                sd = pp.tile([128, 2], F32, tag=f"sd{tag}", name=f"sd{tag}")
                rs = pp.tile([128, 2], F32, tag=f"rs{tag}", name=f"rs{tag}")
                sc = pp.tile([128, 2], F32, tag=f"sc{tag}", name=f"sc{tag}")
                msc = pp.tile([128, 2], F32, tag=f"msc{tag}", name=f"msc{tag}")
                bi = pp.tile([128, 2], F32, tag=f"bi{tag}", name=f"bi{tag}")
                nc.vector.tensor_scalar_mul(mu[:, :], ar_out[:, 0:2], inv_cnt)
                nc.vector.tensor_scalar_mul(ex2[:, :], ar_out[:, 2:4], inv_cnt)
                nc.vector.tensor_tensor(var[:, :], mu[:, :], mu[:, :], OP.mult)
                nc.vector.tensor_tensor(var[:, :], ex2[:, :], var[:, :],
                                        OP.subtract)
                epst = pp.tile([128, 1], F32, tag=f"eps{tag}", name=f"eps{tag}")
                nc.vector.memset(epst[:, :], EPS_BN)
                nc.scalar.activation(sd[:, :], var[:, :], AF.Sqrt,
                                     bias=epst[:, :])
                nc.vector.reciprocal(rs[:, :], sd[:, :])
                nc.vector.tensor_tensor(sc[:, :], rs[:, :], gam[:, :], OP.mult)
                nc.vector.tensor_tensor(msc[:, :], mu[:, :], sc[:, :], OP.mult)
                nc.vector.tensor_tensor(bi[:, :], bet[:, :], msc[:, :],
                                        OP.subtract)
                return sc, bi

            sc1, bi1 = bn_scale_bias(ar_out1, gam0, bet0, "1")
            for t in range(2):
                nc.scalar.activation(
                    out1[t][:, :], out1[t][:, :], AF.Relu,
                    bias=bi1[:, t:t + 1], scale=sc1[:, t:t + 1],
                )

            if level < 99:
                nc.sync.dma_start(out=out_d[0:128, :], in_=out1[0][:, :])
                return

            # ---------------- phase C: MLP2 + BN2 + out ----------------
            with (
                tc.tile_pool(name="pC", bufs=1) as pc,
                tc.tile_pool(name="pCq", bufs=2) as pcq,
                tc.tile_pool(name="pm2", bufs=2, space="PSUM") as pm2_pool,
            ):
                y2 = [pc.tile([128, N], F32, tag=f"y2_{t}", name=f"y2_{t}") for t in range(2)]
                for g in range(NGRP):
                    nsl = slice(512 * g, 512 * (g + 1))
                    for ot in range(2):
                        osl = slice(128 * ot, 128 * (ot + 1))
                        pm = pm2_pool.tile([128, 512], F32, tag="pm2", name="pm2")
                        nc.tensor.matmul(pm[:, :], lhsT=w1a[:, osl],
                                         rhs=out1[0][:, nsl],
                                         start=True, stop=False)
                        nc.tensor.matmul(pm[:, :], lhsT=w1b[:, osl],
                                         rhs=out1[1][:, nsl],
                                         start=False, stop=True)
                        nc.scalar.activation(
                            y2[ot][:, nsl], pm[:, :], AF.Copy,
                            accum_out=s2[ot][:, g:g + 1],
                        )
                        sqd = pcq.tile([128, 512], F32, tag="sqd2", name="sqd2")
                        nc.scalar.activation(
                            sqd[:, :], pm[:, :], AF.Square,
                            accum_out=s2q[ot][:, g:g + 1],
                        )

                ar_in2 = pp.tile([128, 4], F32, tag="ar_in2")
                ar_out2 = pp.tile([128, 4], F32, tag="ar_out2")
                for t in range(2):
                    nc.vector.tensor_reduce(ar_in2[:, t:t + 1], s2[t][:, :],
                                            axis=AX.X, op=OP.add)
                    nc.vector.tensor_reduce(ar_in2[:, 2 + t:3 + t],
                                            s2q[t][:, :], axis=AX.X, op=OP.add)
                bnc_i2 = dp.tile([128, 4], F32, tag="bnc_i2")
                bnc_o2 = dp.tile([128, 4], F32, tag="bnc_o2")
                nc.sync.dma_start(out=bnc_i2[:, :], in_=ar_in2[:, :])
                nc.gpsimd.collective_compute(
                    "AllReduce", OP.add, replica_groups=groups,
                    ins=[bnc_i2[:, :].opt()], outs=[bnc_o2[:, :].opt()],
                )
                nc.sync.dma_start(out=ar_out2[:, :], in_=bnc_o2[:, :])
                sc2, bi2 = bn_scale_bias(ar_out2, gam1, bet1, "2")
                for t in range(2):
                    nc.scalar.activation(
                        y2[t][:, :], y2[t][:, :], AF.Relu,
                        bias=bi2[:, t:t + 1], scale=sc2[:, t:t + 1],
                    )
                    nc.sync.dma_start(
                        out=out_d[128 * t:128 * (t + 1), :], in_=y2[t][:, :]
                    )


def make_in_maps(xyz_pc1, xyz_pc2, points1, points2,
                 W0, b0, gamma0, beta0, W1, b1, gamma1, beta1,
                 n_cores=8):
    # Conv bias b0/b1 cancels exactly in train-mode BatchNorm; unused.
    del b0, b1
    w0t = np.ascontiguousarray(W0.T.astype(np.float32))
    w1t = np.ascontiguousarray(W1.T.astype(np.float32))
    g0 = np.ascontiguousarray(gamma0.reshape(2, 128).T.astype(np.float32))
    be0 = np.ascontiguousarray(beta0.reshape(2, 128).T.astype(np.float32))
    g1 = np.ascontiguousarray(gamma1.reshape(2, 128).T.astype(np.float32))
    be1 = np.ascontiguousarray(beta1.reshape(2, 128).T.astype(np.float32))
    ident = np.eye(128, dtype=np.float32)
    in_maps = []
    for b in range(n_cores):
        x1 = xyz_pc1[b].astype(np.float32)
        x2 = xyz_pc2[b].astype(np.float32)
        a_aug = np.concatenate(
            [x1, x1 * x1, np.full((3, x1.shape[1]), -1.0, np.float32)], axis=0)
        b_aug = np.concatenate(
            [2.0 * x2, np.full((3, x2.shape[1]), -1.0, np.float32), x2 * x2],
            axis=0)
        in_maps.append({
            "a_aug": np.ascontiguousarray(a_aug),
            "b_aug": np.ascontiguousarray(b_aug),
            "p1feat": np.ascontiguousarray(points1[b].astype(np.float32)),
            "f2": np.ascontiguousarray(points2[b].T.astype(np.float32)),
            "W0T": w0t, "W1T": w1t,
            "g0": g0, "be0": be0, "g1": g1, "be1": be1,
            "ident": ident,
        })
    return in_maps


def kernel(**inputs) -> np.ndarray:
    global LAST_RESULTS
    in_maps = make_in_maps(
        inputs["xyz_pc1"], inputs["xyz_pc2"],
        inputs["points1"], inputs["points2"],
        inputs["W0"], inputs["b0"], inputs["gamma0"], inputs["beta0"],
        inputs["W1"], inputs["b1"], inputs["gamma1"], inputs["beta1"],
        n_cores=B,
    )
    nc = build_nc(n_cores=B, debug=False)
    res = run_bass_kernel_spmd(nc, in_maps, core_ids=list(range(B)))
    LAST_RESULTS = res
    out = np.stack([res.results[b]["out"] for b in range(B)], axis=0)
    return out.astype(np.float32)
